# revision 15
# baseline (speedup 1.0000x reference)
"""Trainium2 Bass kernel for nn_ClusteringLoss (k-means, K=64, D=128, 10 iters).

Strategy (sharding_hint): shard Z row-wise across 8 cores; replicate
centroids; per-core distance matmuls + one-hot segment-sum matmuls on the
tensor engine; AllReduce the [K, D+1] sums|counts each iteration.

Per-core layout: Z rows resident in SBUF as bf16, [row, D] layout with a
baked-in ones column per 128-row tile (129 cols/tile, ~126 KB/partition).
Each iteration, per 128-row tile:
  - DMA-xbar-transpose the bf16 tile -> zt [D, rows] (SBUF->SBUF)
  - PE: psum_d[g] = c2 (3-split bf16 prefill matmul) + zt.T @ (-2*C^T)
  - DVE: dmin = min over K (tensor_reduce; 8 tiles share one PSUM bank)
  - ACT: broadcast dmin back to [rows, K]
  - DVE: A = is_equal(psum_d, dmin_rep)  (one-hot, bf16)
  - PE: sums|counts += A.T @ [z_tile | 1]   [K, D+1] PSUM accumulate
AllReduce([K, D+1]) -> centroid update on-device (1/max(count, 0.5)
reproduces the reference's 0/0->0 empty-cluster rule exactly, since an
all-zero one-hot column yields exactly-zero sums).
Final pass computes labels (iota dot one-hot) and min-distances (+|x|^2,
accumulated at load time via Square activation) for the loss.

Numerics: distances use bf16 inputs (products exact, fp32 PSUM accumulate),
c2 exact to ~1e-7 via 3-way bf16 split.  K-means labels on this data are
chaotic (any fp deviation amplifies ~100x/iter; even an exact fp64 replica
of the reference disagrees on ~1.6% of labels), so labels match to ~0.1
L2-rel — the attainable floor; the loss matches to ~3e-7.
"""
import numpy as np
import ml_dtypes

N, D, K, NITER, NCORES = 500000, 128, 64, 10, 8
G = 8       # tiles per PSUM bank (macro-tile)
TC = D + 1  # columns per tile in the residency buffer (z | 1)

bf16np = ml_dtypes.bfloat16


# ---------------------------------------------------------------- bass build
def build_nc(nmacros, nrem, niter, ncores, unroll=4):
    """nmacros macro-tiles of G*128 rows each, plus one remainder tile of
    nrem rows (nrem in 1..128).  Total rows = nmacros*G*128 + nrem."""
    import concourse.bass as bass
    import concourse.bacc as bacc
    import concourse.mybir as mybir
    import concourse.tile as tile

    f32 = mybir.dt.float32
    bf = mybir.dt.bfloat16
    i32 = mybir.dt.int32
    AF = mybir.ActivationFunctionType
    OP = mybir.AluOpType
    AX = mybir.AxisListType
    ds, ts = bass.ds, bass.ts

    nfull = nmacros * G
    ntiles = nfull + 1          # incl. remainder tile
    rows = nfull * 128 + nrem

    nc = bacc.Bacc("TRN2", target_bir_lowering=False, debug=False,
                   num_devices=ncores)

    # ---- DRAM I/O
    z_in = nc.dram_tensor("z", [rows, D], f32, kind="ExternalInput")
    ct2_in = nc.dram_tensor("ct2hi0", [D, K], bf, kind="ExternalInput")
    c2rep_in = nc.dram_tensor("c2rep0", [3, G * K], bf, kind="ExternalInput")
    ones3_in = nc.dram_tensor("ones3", [3, 128], bf, kind="ExternalInput")
    ones1_in = nc.dram_tensor("ones1", [1, 128], f32, kind="ExternalInput")
    onescol_f32_in = nc.dram_tensor("onescol_f32", [128, 1], f32,
                                    kind="ExternalInput")
    zeros129_in = nc.dram_tensor("zeros129", [1, TC], f32, kind="ExternalInput")
    iota8_in = nc.dram_tensor("iota8", [128, G * K], bf, kind="ExternalInput")
    i64_in = nc.dram_tensor("i64", [K, K], bf, kind="ExternalInput")
    cl_out = nc.dram_tensor("cl_out", [ntiles * 128], i32, kind="ExternalOutput")
    loss_out = nc.dram_tensor("loss_part", [1, 1], f32, kind="ExternalOutput")

    # ---- persistent SBUF
    zres = nc.alloc_sbuf_tensor("zres", [128, ntiles * TC], bf).ap()
    x2sb = nc.alloc_sbuf_tensor("x2sb", [128, ntiles], f32).ap()
    clbuf = nc.alloc_sbuf_tensor("clbuf", [128, ntiles], i32).ap()
    lossbuf = nc.alloc_sbuf_tensor("lossbuf", [128, ntiles], f32).ap()
    ct2 = nc.alloc_sbuf_tensor("ct2", [D, K], bf).ap()           # -2*C^T bf16
    c2row = nc.alloc_sbuf_tensor("c2row", [3, G * K], bf).ap()   # c2 splits x8
    ones3 = nc.alloc_sbuf_tensor("ones3_sb", [3, 128], bf).ap()
    ones1 = nc.alloc_sbuf_tensor("ones1_sb", [1, 128], f32).ap()
    onescol_f32 = nc.alloc_sbuf_tensor("onescol_f32_sb", [128, 1], f32).ap()
    zeros129 = nc.alloc_sbuf_tensor("zeros129_sb", [1, TC], f32).ap()
    iota8 = nc.alloc_sbuf_tensor("iota8_sb", [128, G * K], bf).ap()
    i64sb = nc.alloc_sbuf_tensor("i64_sb", [K, K], bf).ap()
    ssb = nc.alloc_sbuf_tensor("ssb", [K, TC], f32).ap()         # sums|counts
    cntcol = nc.alloc_sbuf_tensor("cntcol", [K, 1], f32).ap()
    reccol = nc.alloc_sbuf_tensor("reccol", [K, 1], f32).ap()
    cf = nc.alloc_sbuf_tensor("cf", [K, D], f32).ap()            # centroids f32
    ctm2bf = nc.alloc_sbuf_tensor("ctm2bf", [K, D], bf).ap()     # bf16(-2*C)
    sqf = nc.alloc_sbuf_tensor("sqf", [K, D], f32).ap()
    c2col = nc.alloc_sbuf_tensor("c2col", [K, 1], f32).ap()
    c2pack = nc.alloc_sbuf_tensor("c2pack", [K, 3], bf).ap()
    c2t1 = nc.alloc_sbuf_tensor("c2t1", [K, 1], f32).ap()
    c2t2 = nc.alloc_sbuf_tensor("c2t2", [K, 1], f32).ap()
    losscol = nc.alloc_sbuf_tensor("losscol", [128, 1], f32).ap()
    losssc = nc.alloc_sbuf_tensor("losssc", [1, 1], f32).ap()

    # ---- persistent PSUM
    ps_s = nc.alloc_psum_tensor("ps_s", [K, TC], f32).ap()       # sums|counts
    ps_c2t = nc.alloc_psum_tensor("ps_c2t", [3, K], f32).ap()
    ps_loss = nc.alloc_psum_tensor("ps_loss", [1, 1], f32).ap()

    with tile.TileContext(nc) as tc:
        with (
            tc.tile_pool(name="stage", bufs=3) as stage_pool,
            tc.tile_pool(name="zt", bufs=2 * G) as zt_pool,
            tc.tile_pool(name="wk", bufs=2) as wk_pool,
            tc.tile_pool(name="psd", bufs=2, space="PSUM") as psd_pool,
            tc.tile_pool(name="dram", bufs=2, space="DRAM") as dram_pool,
        ):
            # ================= pre-pass: consts + residency ================
            nc.sync.dma_start(ct2[:, :], ct2_in.ap())
            nc.sync.dma_start(c2row[:, :], c2rep_in.ap())
            nc.sync.dma_start(ones3[:, :], ones3_in.ap())
            nc.sync.dma_start(ones1[:, :], ones1_in.ap())
            nc.sync.dma_start(onescol_f32[:, :], onescol_f32_in.ap())
            nc.sync.dma_start(zeros129[:, :], zeros129_in.ap())
            nc.sync.dma_start(iota8[:, :], iota8_in.ap())
            nc.sync.dma_start(i64sb[:, :], i64_in.ap())
            nc.vector.memset(lossbuf[:, :], 0.0)
            nc.vector.memset(zres[:, ds(nfull * TC, TC)], 0.0)
            # ones column per tile (col 128 of each 129-col tile block)
            nc.vector.memset(
                zres[:, :].rearrange("p (t c) -> p t c", c=TC)[:, :, D:], 1.0)

            z2d = z_in.ap()  # [rows, D]

            def prep_tile(i, nr=128):
                st = stage_pool.tile([128, D], f32, tag="stage")
                nc.sync.dma_start(st[:nr, :], z2d[ds(i * 128, nr), :])
                nc.scalar.activation(zres[:nr, ds(i * TC, D)], st[:nr, :],
                                     AF.Copy)
                junk = stage_pool.tile([128, D], f32, tag="junk")
                nc.scalar.activation(junk[:nr, :], st[:nr, :], AF.Square,
                                     accum_out=x2sb[:nr, ds(i, 1)])

            tc.For_i_unrolled(0, nfull, 1, prep_tile, max_unroll=8)
            prep_tile(nfull, nrem)

            # ================= iterations ==================================
            for it in range(niter + 1):
                final = it == niter

                if not final:
                    # zero the stat accumulator (fp32 matmul writes zeros)
                    nc.tensor.matmul(ps_s[:, :], lhsT=ones1[:, :K],
                                     rhs=zeros129[:, :], start=True,
                                     stop=False)

                def tile_head(base, nr, psd, goff):
                    """transpose + distance matmuls for one 128-row tile"""
                    zt = zt_pool.tile([128, 128], bf, tag="zt")
                    nc.sync.dma_start_transpose(zt[:, :], zres[:, ds(base, 128)])
                    nc.tensor.matmul(psd[:nr, ts(goff, K)], lhsT=zt[:, :nr],
                                     rhs=ct2[:, :], start=False, stop=True)
                    return zt

                def macro(mi):
                    mbase = mi * (G * TC)
                    psd = psd_pool.tile([128, G * K], f32, tag="psd")
                    nc.tensor.matmul(psd[:, :], lhsT=ones3[:, :],
                                     rhs=c2row[:, :], start=True, stop=False)
                    for g in range(G):
                        tile_head(mbase + g * TC, 128, psd, g)
                    dmin8 = wk_pool.tile([128, G], f32, tag="dmin8")
                    psd3 = psd[:, :].rearrange("p (g k) -> p g k", k=K)
                    nc.vector.tensor_reduce(dmin8[:, :], psd3, axis=AX.X,
                                            op=OP.min)
                    drep = wk_pool.tile([128, G * K], f32, tag="drep")
                    rep_in = dmin8[:, :].rearrange("p (g o) -> p g o", o=1) \
                        .broadcast_to((128, G, K))
                    nc.scalar.activation(
                        drep[:, :].rearrange("p (g k) -> p g k", k=K),
                        rep_in, AF.Copy)
                    Aoh = wk_pool.tile([128, G * K], bf, tag="Aoh")
                    nc.vector.tensor_tensor(Aoh[:, :], psd[:, :], drep[:, :],
                                            op=OP.is_equal)
                    if not final:
                        for g in range(G):
                            nc.tensor.matmul(
                                ps_s[:, :], lhsT=Aoh[:, ts(g, K)],
                                rhs=zres[:, ds(mbase + g * TC, TC)],
                                start=False, stop=False)
                    else:
                        g2 = wk_pool.tile([128, G * K], f32, tag="g2")
                        nc.vector.tensor_tensor(g2[:, :], Aoh[:, :],
                                                iota8[:, :], op=OP.mult)
                        clf = wk_pool.tile([128, G], f32, tag="clf")
                        nc.vector.tensor_reduce(
                            clf[:, :],
                            g2[:, :].rearrange("p (g k) -> p g k", k=K),
                            axis=AX.X, op=OP.add)
                        nc.vector.tensor_copy(clbuf[:, ds(mi * G, G)],
                                              clf[:, :])
                        nc.vector.tensor_tensor(lossbuf[:, ds(mi * G, G)],
                                                dmin8[:, :],
                                                x2sb[:, ds(mi * G, G)],
                                                op=OP.add)

                tc.For_i_unrolled(0, nmacros, 1, macro, max_unroll=unroll)

                # ---- remainder tile (nrem rows), static python emission
                nr = nrem
                rbase = nfull * TC
                psd = psd_pool.tile([128, G * K], f32, tag="psd")
                nc.tensor.matmul(psd[:nr, :K], lhsT=ones3[:, :nr],
                                 rhs=c2row[:, :K], start=True, stop=False)
                tile_head(rbase, nr, psd, 0)
                dmin8 = wk_pool.tile([128, G], f32, tag="dmin8")
                nc.vector.tensor_reduce(dmin8[:nr, :1], psd[:nr, :K],
                                        axis=AX.X, op=OP.min)
                drep = wk_pool.tile([128, G * K], f32, tag="drep")
                nc.scalar.activation(
                    drep[:nr, :K],
                    dmin8[:nr, :1].broadcast_to((nr, K)), AF.Copy)
                Aoh = wk_pool.tile([128, G * K], bf, tag="Aoh")
                nc.vector.tensor_tensor(Aoh[:nr, :K], psd[:nr, :K],
                                        drep[:nr, :K], op=OP.is_equal)
                if not final:
                    nc.tensor.matmul(ps_s[:, :], lhsT=Aoh[:nr, :K],
                                     rhs=zres[:nr, ds(rbase, TC)],
                                     start=False, stop=True)
                else:
                    g2 = wk_pool.tile([128, G * K], f32, tag="g2")
                    nc.vector.tensor_tensor(g2[:nr, :K], Aoh[:nr, :K],
                                            iota8[:nr, :K], op=OP.mult)
                    clf = wk_pool.tile([128, G], f32, tag="clf")
                    nc.vector.tensor_reduce(clf[:nr, :1], g2[:nr, :K],
                                            axis=AX.X, op=OP.add)
                    nc.vector.tensor_copy(clbuf[:nr, ds(nfull, 1)],
                                          clf[:nr, :1])
                    nc.vector.tensor_tensor(lossbuf[:nr, ds(nfull, 1)],
                                            dmin8[:nr, :1],
                                            x2sb[:nr, ds(nfull, 1)],
                                            op=OP.add)
                    break

                # ---- all-reduce sums|counts  [K, D+1]
                nc.scalar.activation(ssb[:, :], ps_s[:, :], AF.Copy)
                cc_in = dram_pool.tile([K, TC], f32, tag="cc_in")
                cc_out = dram_pool.tile([K, TC], f32, tag="cc_out")
                nc.gpsimd.dma_start(cc_in[:, :], ssb[:, :])
                nc.gpsimd.collective_compute(
                    "AllReduce", mybir.AluOpType.add,
                    replica_groups=[list(range(ncores))],
                    ins=[cc_in.opt()], outs=[cc_out.opt()])
                nc.gpsimd.dma_start(ssb[:, :], cc_out[:, :])

                # ---- centroid update: C = sums * (1/max(cnt,0.5)) ---------
                nc.vector.tensor_scalar_max(cntcol[:, :], ssb[:, D:TC], 0.5)
                nc.vector.reciprocal(reccol[:, :], cntcol[:, :])
                nc.vector.tensor_scalar_mul(cf[:, :], ssb[:, :D],
                                            reccol[:, :])
                nc.vector.tensor_scalar_mul(ctm2bf[:, :], cf[:, :], -2.0)
                nc.sync.dma_start_transpose(ct2[:, :], ctm2bf[:, :])
                nc.scalar.activation(sqf[:, :], cf[:, :], AF.Square)
                nc.vector.tensor_reduce(c2col[:, :], sqf[:, :], axis=AX.X,
                                        op=OP.add)
                # 3-way bf16 split of c2 (hi/lo/r2) then transpose via PE
                nc.vector.tensor_copy(c2pack[:, 0:1], c2col[:, :])
                nc.vector.tensor_copy(c2t1[:, :], c2pack[:, 0:1])
                nc.vector.tensor_tensor(c2t2[:, :], c2col[:, :], c2t1[:, :],
                                        op=OP.subtract)
                nc.vector.tensor_copy(c2pack[:, 1:2], c2t2[:, :])
                nc.vector.tensor_copy(c2t1[:, :], c2pack[:, 1:2])
                nc.vector.tensor_tensor(c2t2[:, :], c2t2[:, :], c2t1[:, :],
                                        op=OP.subtract)
                nc.vector.tensor_copy(c2pack[:, 2:3], c2t2[:, :])
                nc.tensor.matmul(ps_c2t[:, :], lhsT=c2pack[:, :],
                                 rhs=i64sb[:, :], start=True, stop=True)
                nc.scalar.activation(c2row[:, :K], ps_c2t[:, :], AF.Copy)
                nc.vector.tensor_copy(
                    c2row[:, :].rearrange("p (g k) -> p g k", k=K)[:, 1:G, :],
                    c2row[:, :K].rearrange("p (o k) -> p o k", o=1)
                    .broadcast_to((3, G - 1, K)))

            # ================= outputs =====================================
            nc.vector.tensor_reduce(losscol[:, :], lossbuf[:, :], axis=AX.X,
                                    op=OP.add)
            nc.tensor.matmul(ps_loss[:, :], lhsT=losscol[:, :],
                             rhs=onescol_f32[:, :], start=True, stop=True)
            nc.scalar.activation(losssc[:, :], ps_loss[:, :], AF.Copy)
            nc.sync.dma_start(loss_out.ap(), losssc[:, :])
            with nc.allow_non_contiguous_dma(
                    reason="one-time 250KB strided label writeback"):
                nc.sync.dma_start(
                    cl_out.ap().rearrange("(t p) -> p t", p=128), clbuf[:, :])

    nc.finalize()
    return nc


# ---------------------------------------------------------------- host side
def _host_inputs(Z_shard, c0, nmacros, nrem):
    """Build the per-core input map."""
    rows = nmacros * G * 128 + nrem
    assert Z_shard.shape == (rows, D)
    ct = c0.T.astype(np.float32)                       # [D, K]
    ct2hi0 = (-2.0 * ct).astype(bf16np)
    c2 = (ct * ct).sum(0, dtype=np.float32)            # [K]
    c2hi = c2.astype(bf16np)
    r = c2 - c2hi.astype(np.float32)
    c2lo = r.astype(bf16np)
    c2r2 = (r - c2lo.astype(np.float32)).astype(bf16np)
    c2rep0 = np.stack([np.tile(x, G) for x in (c2hi, c2lo, c2r2)])
    iota = np.arange(K, dtype=np.float32)
    iota8 = np.broadcast_to(np.tile(iota, G)[None, :], (128, G * K))
    return {
        "z": np.ascontiguousarray(Z_shard, np.float32),
        "ct2hi0": ct2hi0,
        "c2rep0": np.ascontiguousarray(c2rep0, bf16np),
        "ones3": np.ones((3, 128), bf16np),
        "ones1": np.ones((1, 128), np.float32),
        "onescol_f32": np.ones((128, 1), np.float32),
        "zeros129": np.zeros((1, TC), np.float32),
        "iota8": iota8.astype(bf16np),
        "i64": np.eye(K, dtype=bf16np),
    }


def _initial_centroids(Z):
    import jax
    with jax.default_device(jax.local_devices(backend="cpu")[0]):
        perm = np.asarray(jax.random.permutation(jax.random.key(1), Z.shape[0]))
    return Z[perm[:K]].astype(np.float32)


def kernel(Z):
    import os
    from concourse.bass_utils import run_bass_kernel_spmd

    Z = np.asarray(Z, np.float32)
    assert Z.shape == (N, D)
    c0 = _initial_centroids(Z)

    nsh = N // NCORES                 # 62500
    nmacros, nrem = 61, 36            # 61*8*128 + 36 = 62500
    nc = build_nc(nmacros, nrem, NITER, NCORES,
                  unroll=int(os.environ.get("KM_UNROLL", "4")))

    in_maps = []
    for c in range(NCORES):
        shard = Z[c * nsh:(c + 1) * nsh]
        in_maps.append(_host_inputs(shard, c0, nmacros, nrem))

    res = run_bass_kernel_spmd(nc, in_maps, core_ids=list(range(NCORES)),
                               trace=bool(os.environ.get("KM_TRACE")))
    kernel.last_results = res

    cl = np.concatenate(
        [res.results[c]["cl_out"][:nsh] for c in range(NCORES)])
    loss_sum = sum(float(res.results[c]["loss_part"][0, 0])
                   for c in range(NCORES))
    loss = np.float32(loss_sum / N)
    return loss, cl.astype(np.int32)
